# revision 3
# baseline (speedup 1.0000x reference)
"""Causal self-attention with RoPE on 8 Trainium2 NeuronCores (Bass/Tile).

Self-contained: builds an SPMD Bass kernel, shards the full inputs across the
8 cores as (batch b = core//2, head-half s = core%2), runs via PJRT, and
re-assembles the full [4, 2048, 1024] output (sum of the two proj partials
per batch).

Problem shapes (hardcoded): B=4, T=2048, C=1024, H=16, D=64.
"""
import sys
sys.path.insert(0, "/opt/trn_rl_repo")
import numpy as np
from contextlib import ExitStack

import concourse.bass as bass
import concourse.bacc as bacc
import concourse.mybir as mybir
import concourse.tile as tile

F32 = mybir.dt.float32
F32R = mybir.dt.float32r
AF = mybir.ActivationFunctionType

B = 4
T = 2048
N_EMBD = 1024
D = 64
HL = 8                      # local heads per core
CL = HL * D                 # 512
CT = CL // 128              # 4
VW = HL * 65                # 520
CIN_TILES = N_EMBD // 128   # 8
N_CORES = 8


# ====================== device program ======================

def _build_nc():
    NQT = T // 512
    NKT = T // 128
    NCH = T // 512

    nc = bacc.Bacc("TRN2", target_bir_lowering=False, debug=False,
                   num_devices=N_CORES)

    xT = nc.declare_dram_parameter("xT", [N_EMBD, T], F32, isOutput=False)
    wq = nc.declare_dram_parameter("wq", [N_EMBD, CL], F32, isOutput=False)
    wk = nc.declare_dram_parameter("wk", [N_EMBD, CL], F32, isOutput=False)
    wv = nc.declare_dram_parameter("wv", [N_EMBD, VW], F32, isOutput=False)
    projw = nc.declare_dram_parameter("projw", [CL, N_EMBD], F32, isOutput=False)
    r2 = nc.declare_dram_parameter("r2", [128, 128], F32, isOutput=False)
    cosT = nc.declare_dram_parameter("cosT", [128, T], F32, isOutput=False)
    sinT = nc.declare_dram_parameter("sinT", [128, T], F32, isOutput=False)
    qkbias = nc.declare_dram_parameter("qkbias", [128, 2 * CT], F32, isOutput=False)
    bv = nc.declare_dram_parameter("bv", [1, VW], F32, isOutput=False)
    pbh = nc.declare_dram_parameter("pbh", [1, N_EMBD], F32, isOutput=False)
    ones = nc.declare_dram_parameter("ones", [1, 128], F32, isOutput=False)
    masks = nc.declare_dram_parameter("masks", [128, 2 * 1024], F32, isOutput=False)
    out = nc.declare_dram_parameter("out", [T, N_EMBD], F32, isOutput=True)

    with tile.TileContext(nc) as tc, ExitStack() as ctx:
        res = ctx.enter_context(tc.tile_pool(name="res", bufs=1))
        qt_tiles = [res.tile([128, T], F32R, tag=f"qt{i}", name=f"qt{i}")
                    for i in range(CT)]
        kt_tiles = [res.tile([128, T], F32R, tag=f"kt{i}", name=f"kt{i}")
                    for i in range(CT)]
        v_tiles = [res.tile([128, VW], F32R, tag=f"v{j}", name=f"v{j}")
                   for j in range(NKT)]
        ones_sb = res.tile([1, 128], F32R, tag="ones")
        nc.sync.dma_start(ones_sb[:], ones[:].bitcast(F32R))

        # ---------------- phase 1: QKV + RoPE ----------------
        with ExitStack() as p1:
            wpool = p1.enter_context(tc.tile_pool(name="wpool", bufs=1))
            wq_sb = wpool.tile([128, CIN_TILES * CL], F32R, tag="wq")
            wk_sb = wpool.tile([128, CIN_TILES * CL], F32R, tag="wk")
            wv_sb = wpool.tile([128, CIN_TILES * VW], F32R, tag="wv")
            r2_sb = wpool.tile([128, 128], F32R, tag="r2")
            qkb_sb = wpool.tile([128, 2 * CT], F32, tag="qkb")
            bv_sb = wpool.tile([1, VW], F32R, tag="bv")
            # first x-chunk loads before weights so PE starts sooner
            xpool0 = p1.enter_context(tc.tile_pool(name="xpool", bufs=2))
            xc_first = xpool0.tile([128, CIN_TILES * 512], F32R, tag="xc",
                                   name="xc_first")
            nc.sync.dma_start(
                xc_first.rearrange("p (a t) -> p a t", a=CIN_TILES),
                xT[:, 0:512].rearrange("(a p) t -> p a t", p=128).bitcast(F32R))
            nc.sync.dma_start(
                wq_sb.rearrange("p (a c) -> p a c", a=CIN_TILES),
                wq[:].rearrange("(a p) c -> p a c", p=128).bitcast(F32R))
            nc.sync.dma_start(
                wk_sb.rearrange("p (a c) -> p a c", a=CIN_TILES),
                wk[:].rearrange("(a p) c -> p a c", p=128).bitcast(F32R))
            nc.sync.dma_start(
                wv_sb.rearrange("p (a c) -> p a c", a=CIN_TILES),
                wv[:].rearrange("(a p) c -> p a c", p=128).bitcast(F32R))
            nc.sync.dma_start(r2_sb[:], r2[:].bitcast(F32R))
            nc.sync.dma_start(qkb_sb[:], qkbias[:])
            nc.sync.dma_start(bv_sb[:], bv[:].bitcast(F32R))

            xpool = xpool0
            cspool = p1.enter_context(tc.tile_pool(name="cspool", bufs=2))
            qkps = p1.enter_context(tc.tile_pool(name="qkps", bufs=2, space="PSUM"))
            rotps = p1.enter_context(tc.tile_pool(name="rotps", bufs=2, space="PSUM"))
            vps = p1.enter_context(tc.tile_pool(name="vps", bufs=2, space="PSUM"))
            evict = p1.enter_context(tc.tile_pool(name="evict", bufs=2))

            for j in range(NCH):
                tj = slice(j * 512, (j + 1) * 512)
                if j == 0:
                    xc = xc_first
                else:
                    xc = xpool.tile([128, CIN_TILES * 512], F32R, tag="xc",
                                    name="xc")
                    nc.sync.dma_start(
                        xc.rearrange("p (a t) -> p a t", a=CIN_TILES),
                        xT[:, tj].rearrange("(a p) t -> p a t", p=128)
                        .bitcast(F32R))
                cos_c = cspool.tile([128, 512], F32, tag="cos")
                sin_c = cspool.tile([128, 512], F32, tag="sin")
                nc.sync.dma_start(cos_c[:], cosT[:, tj])
                nc.sync.dma_start(sin_c[:], sinT[:, tj])

                for which, w_sb, t_dst in ((0, wq_sb, qt_tiles),
                                           (1, wk_sb, kt_tiles)):
                    for i in range(CT):
                        ps = qkps.tile([128, 512], F32, tag="qk")
                        for a in range(CIN_TILES):
                            nc.tensor.matmul(
                                ps[:],
                                w_sb[:, a * CL + i * 128: a * CL + (i + 1) * 128],
                                xc[:, a * 512:(a + 1) * 512],
                                start=(a == 0), stop=(a == CIN_TILES - 1))
                        qb = evict.tile([128, 512], F32R, tag="qb")
                        bcol = which * CT + i
                        nc.vector.tensor_scalar_add(qb[:], ps[:],
                                                    qkb_sb[:, bcol:bcol + 1])
                        rps = rotps.tile([128, 512], F32, tag="rot")
                        nc.tensor.matmul(rps[:], r2_sb[:], qb[:],
                                         start=True, stop=True)
                        t1 = evict.tile([128, 512], F32, tag="t1")
                        nc.vector.tensor_mul(t1[:], qb[:], cos_c[:])
                        t2 = evict.tile([128, 512], F32, tag="t2")
                        nc.vector.tensor_mul(t2[:], rps[:], sin_c[:])
                        nc.vector.tensor_add(t_dst[i][:, tj], t1[:], t2[:])

                for tt in range(4):
                    vt = v_tiles[j * 4 + tt]
                    for ch in range(2):
                        cw = VW // 2
                        vsl = slice(ch * cw, (ch + 1) * cw)
                        ps = vps.tile([128, cw], F32, tag="v")
                        for a in range(CIN_TILES):
                            nc.tensor.matmul(
                                ps[:],
                                xc[:, a * 512 + tt * 128: a * 512 + (tt + 1) * 128],
                                wv_sb[:, a * VW:(a + 1) * VW][:, vsl],
                                start=(a == 0), stop=False)
                        nc.tensor.matmul(ps[:], ones_sb[:], bv_sb[:, vsl],
                                         start=False, stop=True)
                        nc.scalar.copy(vt[:, vsl], ps[:])

        # ---------------- phases 2+3 ----------------
        with ExitStack() as p23:
            late = p23.enter_context(tc.tile_pool(name="late", bufs=1))
            yt_tiles = [late.tile([128, T], F32R, tag=f"yt{i}", name=f"yt{i}")
                        for i in range(CT)]
            pw_sb = late.tile([128, CT * N_EMBD], F32R, tag="pw")
            pbh_sb = late.tile([1, N_EMBD], F32R, tag="pbh")
            mask_sb = late.tile([128, 2 * 1024], F32, tag="masks")
            nc.sync.dma_start(
                pw_sb.rearrange("p (a c) -> p a c", a=CT),
                projw[:].rearrange("(a p) c -> p a c", p=128).bitcast(F32R))
            nc.sync.dma_start(pbh_sb[:], pbh[:].bitcast(F32R))
            nc.sync.dma_start(mask_sb[:], masks[:])

            with ExitStack() as p2:
                sps = p2.enter_context(
                    tc.tile_pool(name="sps", bufs=3, space="PSUM"))
                yps = p2.enter_context(
                    tc.tile_pool(name="yps", bufs=2, space="PSUM"))
                ppool = p2.enter_context(tc.tile_pool(name="ppool", bufs=4))
                npool = p2.enter_context(tc.tile_pool(name="npool", bufs=3))

                for qi in range(NQT):
                    qs = slice(qi * 512, (qi + 1) * 512)
                    nk = 4 * (qi + 1)
                    for hp in range(HL // 2):          # head pairs (row-tiled)
                        kt_h = kt_tiles[hp]
                        qt_h = qt_tiles[hp]
                        yp0 = yps.tile([65, 512], F32, tag="y", name="yp0")
                        yp1 = yps.tile([65, 512], F32, tag="y", name="yp1")
                        for kp in range(nk // 2):      # paired k-tiles
                            kb0, kb1 = 2 * kp, 2 * kp + 1
                            sp0 = sps.tile([128, 1024], F32, tag="s", name="sp0")
                            sp1 = sps.tile([128, 1024], F32, tag="s", name="sp1")
                            for u, kb in ((0, kb0), (1, kb1)):
                                ks = slice(kb * 128, (kb + 1) * 128)
                                # two heads concurrently in PE row groups
                                nc.tensor.matmul(
                                    sp0[:, u * 512:(u + 1) * 512],
                                    kt_h[0:64, ks], qt_h[0:64, qs],
                                    start=True, stop=True,
                                    tile_position=(0, 0))
                                nc.tensor.matmul(
                                    sp1[:, u * 512:(u + 1) * 512],
                                    kt_h[64:128, ks], qt_h[64:128, qs],
                                    start=True, stop=True,
                                    tile_position=(64, 0))
                            mp = kp - 2 * qi
                            for hh, sp, yp in ((0, sp0, yp0), (1, sp1, yp1)):
                                h = 2 * hp + hh
                                pt = ppool.tile([128, 1024], F32R, tag="pt",
                                                name="pt")
                                nc.scalar.activation(pt[:], sp[:], AF.Exp,
                                                     scale=0.125)
                                if mp >= 0:            # diagonal pair: mask
                                    nc.gpsimd.tensor_mul(
                                        pt[:], pt[:],
                                        mask_sb[:, mp * 1024:(mp + 1) * 1024])
                                for u, kb in ((0, kb0), (1, kb1)):
                                    nc.tensor.matmul(
                                        yp[:],
                                        v_tiles[kb][:, h * 65:h * 65 + 65],
                                        pt[:, u * 512:(u + 1) * 512],
                                        start=(kb == 0), stop=(kb == nk - 1))
                        for hh, yp in ((0, yp0), (1, yp1)):
                            h = 2 * hp + hh
                            pb = hh * 64
                            rec = npool.tile([1, 512], F32, tag="rec",
                                             name="rec")
                            nc.vector.reciprocal(rec[:], yp[64:65, :])
                            bc = npool.tile([64, 512], F32, tag="bc", name="bc")
                            nc.gpsimd.partition_broadcast(bc[:], rec[:])
                            nc.vector.tensor_mul(yt_tiles[hp][pb:pb + 64, qs],
                                                 yp[0:64, :], bc[:])

            with ExitStack() as p3:
                ops_ = p3.enter_context(
                    tc.tile_pool(name="ops", bufs=2, space="PSUM"))
                opool = p3.enter_context(tc.tile_pool(name="opool", bufs=3))
                for tt in range(T // 128):
                    for ch in range(2):
                        cs = slice(ch * 512, (ch + 1) * 512)
                        ps = ops_.tile([128, 512], F32, tag="o")
                        for a in range(CT):
                            nc.tensor.matmul(
                                ps[:],
                                yt_tiles[a][:, tt * 128:(tt + 1) * 128],
                                pw_sb[:, a * N_EMBD:(a + 1) * N_EMBD][:, cs],
                                start=(a == 0), stop=False)
                        nc.tensor.matmul(ps[:], ones_sb[:], pbh_sb[:, cs],
                                         start=False, stop=True)
                        ot = opool.tile([128, 512], F32, tag="ot")
                        nc.scalar.copy(ot[:], ps[:])
                        nc.sync.dma_start(out[tt * 128:(tt + 1) * 128, cs], ot[:])

    nc.compile()
    return nc


# ====================== host-side sharding ======================

def _rope_tables():
    inv_freq = 1.0 / (10000.0 ** (np.arange(0, D, 2, dtype=np.float64) / D))
    t = np.arange(T, dtype=np.float64)
    fr = np.outer(t, inv_freq)
    emb = np.concatenate([fr, fr], axis=-1)
    return np.cos(emb).astype(np.float32), np.sin(emb).astype(np.float32)


def _make_in_maps(x, qkv_w, qkv_b, proj_w, proj_b):
    C = N_EMBD
    cos, sin = _rope_tables()
    cos2 = np.ascontiguousarray(np.vstack([cos.T, cos.T]).astype(np.float32))
    sin2 = np.ascontiguousarray(np.vstack([sin.T, sin.T]).astype(np.float32))
    M = np.zeros((D, D), dtype=np.float32)
    for d in range(32):
        M[d, d + 32] = -1.0
        M[d + 32, d] = 1.0
    R2 = np.zeros((128, 128), dtype=np.float32)
    R2[:64, :64] = M.T
    R2[64:, 64:] = M.T
    msk4 = np.zeros((4, 128, 512), dtype=np.float32)
    p = np.arange(128)[:, None]
    f = np.arange(512)[None, :]
    for dd in range(4):
        msk4[dd] = ((p + dd * 128) <= f).astype(np.float32)
    # paired: [m0|m1] then [m2|m3]
    msk = np.concatenate([msk4[0], msk4[1], msk4[2], msk4[3]], axis=1)
    msk = np.ascontiguousarray(msk)
    ones_np = np.ones((1, 128), dtype=np.float32)
    pbh_np = np.ascontiguousarray((proj_b / 2.0).astype(np.float32)[None, :])

    in_maps = []
    for c in range(N_CORES):
        b, s = divmod(c, 2)
        cl0 = s * CL
        wq_ = np.ascontiguousarray(qkv_w[:, cl0:cl0 + CL])
        wk_ = np.ascontiguousarray(qkv_w[:, C + cl0:C + cl0 + CL])
        wv_raw = qkv_w[:, 2 * C + cl0:2 * C + cl0 + CL]
        bq = qkv_b[cl0:cl0 + CL]
        bk = qkv_b[C + cl0:C + cl0 + CL]
        bv_raw = qkv_b[2 * C + cl0:2 * C + cl0 + CL]
        wv_ = np.zeros((C, VW), dtype=np.float32)
        bv_ = np.zeros((1, VW), dtype=np.float32)
        for h in range(HL):
            wv_[:, 65 * h:65 * h + 64] = wv_raw[:, 64 * h:64 * h + 64]
            bv_[0, 65 * h:65 * h + 64] = bv_raw[64 * h:64 * h + 64]
            bv_[0, 65 * h + 64] = 1.0
        qkb = np.zeros((128, 2 * CT), dtype=np.float32)
        for i in range(CT):
            qkb[:, i] = bq[i * 128:(i + 1) * 128]
            qkb[:, CT + i] = bk[i * 128:(i + 1) * 128]
        in_maps.append({
            "xT": np.ascontiguousarray(x[b].T.astype(np.float32)),
            "wq": wq_, "wk": wk_, "wv": wv_,
            "projw": np.ascontiguousarray(proj_w[cl0:cl0 + CL, :]),
            "r2": R2, "cosT": cos2, "sinT": sin2,
            "qkbias": qkb, "bv": bv_, "pbh": pbh_np,
            "ones": ones_np, "masks": msk,
        })
    return in_maps


# ====================== PJRT runner (jit once) ======================

_CACHE = {}


def _get_runner():
    if "runner" in _CACHE:
        return _CACHE["runner"]
    import jax
    from jax.sharding import Mesh, PartitionSpec, NamedSharding
    from jax.experimental.shard_map import shard_map
    from concourse import bass2jax

    bass2jax.install_neuronx_cc_hook()
    nc = _build_nc()

    partition_name = (nc.partition_id_tensor.name
                      if nc.partition_id_tensor else None)
    in_names, out_names, out_avals, zero_outs = [], [], [], []
    for alloc in nc.m.functions[0].allocations:
        if not isinstance(alloc, mybir.MemoryLocationSet):
            continue
        name = alloc.memorylocations[0].name
        if alloc.kind == "ExternalInput":
            if name != partition_name:
                in_names.append(name)
        elif alloc.kind == "ExternalOutput":
            shape = tuple(alloc.tensor_shape)
            dtype = mybir.dt.np(alloc.dtype)
            out_names.append(name)
            out_avals.append(jax.core.ShapedArray(shape, dtype))
            zero_outs.append(np.zeros(shape, dtype))
    n_params = len(in_names)
    all_in_names = list(in_names) + list(out_names)
    if partition_name is not None:
        all_in_names.append(partition_name)

    def _body(*args):
        operands = list(args)
        if partition_name is not None:
            operands.append(bass2jax.partition_id_tensor())
        outs = bass2jax._bass_exec_p.bind(
            *operands,
            out_avals=tuple(out_avals),
            in_names=tuple(all_in_names),
            out_names=tuple(out_names),
            lowering_input_output_aliases=(),
            sim_require_finite=True,
            sim_require_nnan=True,
            nc=nc,
        )
        return tuple(outs)

    devices = jax.devices()[:N_CORES]
    mesh = Mesh(np.asarray(devices), ("core",))
    n_outs = len(out_names)
    in_specs = (PartitionSpec("core"),) * (n_params + n_outs)
    out_specs = (PartitionSpec("core"),) * n_outs
    sharded = jax.jit(
        shard_map(_body, mesh=mesh, in_specs=in_specs, out_specs=out_specs,
                  check_rep=False),
        keep_unused=True)

    sh = NamedSharding(mesh, PartitionSpec("core"))

    def prepare(in_maps):
        concat_in = [
            np.concatenate([np.asarray(in_maps[c][nm]) for c in range(N_CORES)],
                           axis=0)
            for nm in in_names
        ]
        concat_zeros = [np.zeros((N_CORES * z.shape[0], *z.shape[1:]), z.dtype)
                        for z in zero_outs]
        import jax as _jax
        return [_jax.device_put(a, sh) for a in concat_in + concat_zeros]

    def run(dev_args):
        outs = sharded(*dev_args)
        import jax as _jax
        _jax.block_until_ready(outs)
        return outs

    def fetch(outs):
        res = []
        arr = np.asarray(outs[0]).reshape(N_CORES, *out_avals[0].shape)
        for c in range(N_CORES):
            res.append({out_names[0]: arr[c]})
        return res

    _CACHE["runner"] = (prepare, run, fetch)
    return _CACHE["runner"]


# ====================== public entry point ======================

def kernel(x, qkv_w, qkv_b, proj_w, proj_b):
    prepare, run, fetch = _get_runner()
    in_maps = _make_in_maps(np.asarray(x), np.asarray(qkv_w),
                            np.asarray(qkv_b), np.asarray(proj_w),
                            np.asarray(proj_b))
    dev_args = prepare(in_maps)
    results = fetch(run(dev_args))
    out = np.zeros((B, T, N_EMBD), dtype=np.float32)
    for b in range(B):
        out[b] = results[2 * b]["out"] + results[2 * b + 1]["out"]
    return out
